# revision 26
# baseline (speedup 1.0000x reference)
"""GNN message-passing (GENConv-style, 2 layers x 2 link types) on 8 trn2 cores.

Sharding: partition by destination node range (2500 nodes/core). Each core owns
its nodes' incoming edges for both links/layers. All tables are kept in SLOT
space (per-core bin-packed slot order), so shard outputs are written with plain
contiguous DMAs (no indirect scatters) and AllGathered as bf16 gather tables;
gather indices are precomputed host-side in slot space. Per-block edge-tile
gathers are batched into one indirect DMA (T tiles = T*128 rows) with
compute_op=add onto a pre-filled we*ea buffer. Segment softmax num/den are
one-hot matmuls against host-built S matrices (feature-major). All matmuls run
in bf16; BN scale is folded into W1, BN bias is applied via the Relu
activation's per-partition bias operand. For layer 1 (Identity lin_dst), x1 is
added into agg before the single W1 matmul. Each layer runs as two passes
(link 0 fully, then link 1 + MLP2) so link-0 compute hides link-1's AllGather.
y is written feature-major and un-permuted on the host.
"""

import os

import numpy as np
import ml_dtypes

import concourse.bass as bass
import concourse.mybir as mybir
import concourse.tile as tile
from concourse import bacc
from concourse.bass_utils import run_bass_kernel_spmd
from concourse.masks import make_identity

N_NODES = 20000
FIN = 256
H = 512
H2 = 1024
NCORES = 8
SHARD = N_NODES // NCORES  # 2500
P = 128
NBLK = 20           # slot blocks per core (20*128 = 2560 slots >= 2500)
SLOTS = NBLK * P    # 2560
NCH = SLOTS // H    # 5 slot-chunks of 512
BN_EPS = 1e-5
DEN_EPS = 1e-20

f32 = mybir.dt.float32
bf16 = mybir.dt.bfloat16
i32 = mybir.dt.int32
i16 = mybir.dt.int16
AF = mybir.ActivationFunctionType
OP = mybir.AluOpType

nbf = np.dtype(ml_dtypes.bfloat16)

_cache = {}


def _pack_lhst(wt):
    """[K, M] -> [128, K//128, M//128, 128] so [:, kt, ch, :] is a lhsT tile."""
    K, M = wt.shape
    return np.ascontiguousarray(
        wt.reshape(K // P, P, M // P, P).transpose(1, 0, 2, 3)
    ).astype(nbf)


def _bin_pack(d0, d1):
    """Assign SHARD local nodes to NBLK blocks (<=128 nodes each), balancing
    per-link edge load. Returns list of sorted node-id arrays."""
    d_tot = d0 + d1
    order = np.argsort(-d_tot, kind="stable")
    loads = np.zeros(NBLK, dtype=np.int64)
    counts = np.zeros(NBLK, dtype=np.int64)
    blocks = [[] for _ in range(NBLK)]
    for n in order:
        cand = np.where(counts < P)[0]
        b = cand[np.argmin(loads[cand])]
        blocks[b].append(int(n))
        loads[b] += d_tot[n]
        counts[b] += 1
    return [np.array(sorted(b), dtype=np.int64) for b in blocks]


def _host_prep(ei, ea, lens):
    """Build per-core edge-structure inputs. Returns (T, per_core list)."""
    E = ei.shape[1]
    src_all = ei[0].astype(np.int64)
    dst_all = ei[1].astype(np.int64)
    link0 = np.zeros(E, dtype=bool)
    link0[: lens[0]] = True

    per_core = []
    # slot_global[n] = core(n)*SLOTS + slot_in_core(n)
    slot_global = np.full(N_NODES, -1, dtype=np.int64)
    for c in range(NCORES):
        lo, hi = c * SHARD, (c + 1) * SHARD
        core = {}
        m_core = (dst_all >= lo) & (dst_all < hi)
        dloc_all = dst_all - lo
        d0 = np.bincount(dloc_all[m_core & link0], minlength=SHARD)
        d1 = np.bincount(dloc_all[m_core & ~link0], minlength=SHARD)
        blocks = _bin_pack(d0, d1)

        slot_of_node = np.full(SHARD, -1, dtype=np.int64)
        perm = np.full(SLOTS, SHARD, dtype=np.int64)  # padding -> zero row
        for b, nodes in enumerate(blocks):
            slot_of_node[nodes] = b * P + np.arange(len(nodes))
            perm[b * P: b * P + len(nodes)] = nodes
        assert (slot_of_node >= 0).all()
        core["perm"] = perm
        core["slot_of_node"] = slot_of_node
        slot_global[lo:hi] = c * SLOTS + slot_of_node

        core["links"] = []
        for l in range(2):
            m = m_core & (link0 if l == 0 else ~link0)
            e_idx = np.nonzero(m)[0]
            e_slot = slot_of_node[dst_all[e_idx] - lo]
            e_blk = e_slot // P
            o = np.argsort(e_blk, kind="stable")
            e_idx, e_slot, e_blk = e_idx[o], e_slot[o], e_blk[o]
            cnt = np.bincount(e_blk, minlength=NBLK)
            core["links"].append((e_idx, e_slot, cnt))
        per_core.append(core)

    T = 0
    for core in per_core:
        for (_, _, cnt) in core["links"]:
            T = max(T, int(np.ceil(cnt.max() / P)))

    # x1 table row: chunk-sliced AllGather layout
    cc = slot_global % SLOTS // H            # which slot-chunk (0..4)
    corev = slot_global // SLOTS
    rr = slot_global % SLOTS % H
    x1_row = cc * (NCORES * H) + corev * H + rr

    for core in per_core:
        for l in range(2):
            e_idx, e_slot, cnt = core["links"][l]
            src0 = np.zeros((NBLK, T * P), dtype=np.int64)  # global node id
            eav = np.zeros((NBLK, T * P), dtype=np.float32)
            S = np.zeros((NBLK, T * P, P), dtype=nbf)
            off = 0
            for b in range(NBLK):
                k = int(cnt[b])
                sl = slice(off, off + k)
                src0[b, :k] = src_all[e_idx[sl]]
                eav[b, :k] = ea[e_idx[sl], 0]
                S[b, np.arange(k), e_slot[sl] - b * P] = 1.0
                off += k
            def pack16(rows):
                # rows [NBLK, T*P] in edge order -> [P, NBLK*(T*P//16)]
                # int16, idx i of block b at [i%16 (replicated x8), b*NI + i//16]
                ni = T * P // 16
                out = np.zeros((P, NBLK * ni), np.int16)
                for b in range(NBLK):
                    w = rows[b].reshape(ni, 16).T.astype(np.int16)  # [16, NI]
                    for rep in range(8):
                        out[rep * 16:(rep + 1) * 16, b * ni:(b + 1) * ni] = w
                return out

            core[f"src0_{l}"] = pack16(slot_global[src0])
            core[f"src1_{l}"] = pack16(x1_row[src0])
            core[f"eav_{l}"] = eav.reshape(NBLK, T, P)
            core[f"S_{l}"] = S.reshape(NBLK * T, P, P).copy()
        del core["links"]

    return T, per_core


def _build(T):
    NT = NBLK * T
    nc = bacc.Bacc("TRN2", target_bir_lowering=False, debug=False,
                   num_devices=NCORES)

    def din(name, shape, dt):
        return nc.dram_tensor(name, shape, dt, kind="ExternalInput")

    NI16 = NBLK * (T * P // 16)
    xt_slots = din("xt_slots", [P, 2, SLOTS], bf16)
    edge_d = {}
    for l in range(2):
        edge_d[f"src0_{l}"] = din(f"src0_{l}", [P, NI16], i16)
        edge_d[f"src1_{l}"] = din(f"src1_{l}", [P, NI16], i16)
        edge_d[f"S_{l}"] = din(f"S_{l}", [NT, P, P], bf16)
        for ll in range(2):
            edge_d[f"weea_{ll}_{l}"] = din(f"weea_{ll}_{l}",
                                           [NBLK, P, T * H], bf16)
    wst_d = [din(f"wst_{l}", [P, 2, H], bf16) for l in range(2)]
    bias_d = nc.dram_tensor("bias_all", [P, 32], f32, kind="ExternalInput")
    wd = {}
    for ll in range(2):
        for l in range(2):
            wd[f"w1t_{ll}_{l}"] = din(f"w1t_{ll}_{l}", [P, 4, 8, P], bf16)
            wd[f"w2t_{ll}_{l}"] = din(f"w2t_{ll}_{l}", [P, 8, 4, P], bf16)
    for l in range(2):
        wd[f"wft_0_{l}"] = din(f"wft_0_{l}", [P, 2, 8, P], bf16)
    y_ext = nc.dram_tensor("y_out", [NCH, P, 4 * H], bf16, kind="ExternalOutput")

    sf_bounce = [nc.dram_tensor(f"sf_bounce_{l}", [SLOTS, H], bf16)
                 for l in range(2)]
    sf_table = [nc.dram_tensor(f"sf_table_{l}", [NCORES * SLOTS, H], bf16,
                               addr_space="Shared") for l in range(2)]
    x1_bounce = nc.dram_tensor("x1_bounce", [SLOTS, H], bf16)
    x1_table = nc.dram_tensor("x1_table", [NCORES * SLOTS, H], bf16,
                              addr_space="Shared")

    RG = [list(range(NCORES))]

    with tile.TileContext(nc) as tc:
        with (
            tc.tile_pool(name="const", bufs=1) as cp,
            tc.tile_pool(name="wpool", bufs=1) as wpl,
            tc.tile_pool(name="gzp", bufs=3) as gzp,
            tc.tile_pool(name="spool", bufs=4) as sp,
            tc.tile_pool(name="edge", bufs=2) as ep,
            tc.tile_pool(name="dr", bufs=1) as drp,
            tc.tile_pool(name="agg", bufs=2) as ap_,
            tc.tile_pool(name="h0pool", bufs=NCH) as h0p,
            tc.tile_pool(name="h1pool", bufs=1) as h1p,
            tc.tile_pool(name="x1p", bufs=1) as x1p,
            tc.tile_pool(name="ysb", bufs=1) as ysbp,
            tc.tile_pool(name="scratch", bufs=2) as scr,
            tc.tile_pool(name="pseg", bufs=2, space="PSUM") as pseg,
            tc.tile_pool(name="pall", bufs=2, space="PSUM") as pall,
            tc.tile_pool(name="ptr", bufs=2, space="PSUM") as ptr,
        ):
            def load(pool, dram, shape, dt, tag):
                t = pool.tile(shape, dt, tag=tag, name=tag)
                nc.sync.dma_start(out=t[:], in_=dram.ap())
                return t

            xt_sl = load(cp, xt_slots, [P, 2, SLOTS], bf16, "xt_sl")
            src_t = {}
            for l in range(2):
                src_t[(0, l)] = load(cp, edge_d[f"src0_{l}"], [P, NI16], i16,
                                     f"src0{l}")
                src_t[(1, l)] = load(cp, edge_d[f"src1_{l}"], [P, NI16], i16,
                                     f"src1{l}")
            wst_t = [load(cp, wst_d[l], [P, 2, H], bf16, f"wst{l}")
                     for l in range(2)]
            bias_t = load(cp, bias_d, [P, 32], f32, "bias_all")
            identb = cp.tile([P, P], bf16, tag="identb")
            identf = cp.tile([P, P], f32, tag="identf")
            make_identity(nc, identf[:])
            nc.vector.tensor_copy(out=identb[:], in_=identf[:])

            # ---- sf shards (slot order) + AllGather per link ----
            def sf_phase():
              for l in range(2):
                for b in range(NBLK):
                    ps = pall.tile([P, H], f32, space="PSUM", tag="hp")
                    for kt in range(2):
                        nc.tensor.matmul(
                            out=ps[:],
                            lhsT=xt_sl[:, kt, b * P:(b + 1) * P],
                            rhs=wst_t[l][:, kt, :],
                            start=(kt == 0), stop=(kt == 1))
                    sf_sb = scr.tile([P, H], bf16, tag="scr_bf")
                    nc.vector.tensor_copy(out=sf_sb[:], in_=ps[:])
                    nc.sync.dma_start(
                        out=sf_bounce[l].ap()[b * P:(b + 1) * P, :],
                        in_=sf_sb[:])
                nc.gpsimd.collective_compute(
                    "AllGather", OP.bypass, replica_groups=RG,
                    ins=[sf_bounce[l].ap().opt()],
                    outs=[sf_table[l].ap().opt()])

            sf_phase()
            x1T = x1p.tile([P, 4, SLOTS], bf16, tag="x1T")

            def agg_chunk(ll, l, cchunk, table, m_on_act):
                """Aggregate 4 blocks of cchunk for link l -> aggT tile.

                den/num are computed dst-major ([dst, feat], one stationary
                S-load + two N=512 matmuls per edge tile), then transposed
                back to feature-major via TensorE."""
                aggT = ap_.tile([P, 4, H], bf16, tag=f"aggT{l}",
                                name=f"aggT{l}")
                NI = T * P // 16
                for bi in range(4):
                    b = cchunk * 4 + bi
                    gz = gzp.tile([P, T, H], bf16, tag="gz")
                    nc.gpsimd.dma_gather(
                        gz[:], table.ap(),
                        src_t[(ll, l)][:, b * NI:(b + 1) * NI],
                        T * P, T * P, H)
                    gzf = gz[:].rearrange("p t h -> p (t h)")
                    wa = gzp.tile([P, T * H], bf16, tag="wa")
                    nc.sync.dma_start(
                        out=wa[:], in_=edge_d[f"weea_{ll}_{l}"][b, :, :])
                    # z = gathered + we*ea  (in place into gz)
                    nc.vector.tensor_tensor(
                        out=gzf, in0=gzf, in1=wa[:], op=OP.add)
                    m = ep.tile([P, T * H], bf16, tag="m")
                    nc.vector.tensor_scalar(
                        out=m[:], in0=gzf, scalar1=0.0,
                        scalar2=None, op0=OP.max)
                    pe = ep.tile([P, T * H], bf16, tag="pe")
                    nc.scalar.activation(out=pe[:], in_=m[:], func=AF.Exp)
                    qe = ep.tile([P, T * H], bf16, tag="qe")
                    nc.vector.tensor_tensor(
                        out=qe[:], in0=m[:], in1=pe[:], op=OP.mult)
                    den = pseg.tile([P, H], f32, space="PSUM", tag="den")
                    num = pseg.tile([P, H], f32, space="PSUM", tag="num")
                    for tt in range(T):
                        t_idx = b * T + tt
                        s_tile = sp.tile([P, P], bf16, tag="S")
                        nc.sync.dma_start(
                            out=s_tile[:],
                            in_=edge_d[f"S_{l}"][t_idx, :, :])
                        nc.tensor.matmul(
                            out=den[:], lhsT=s_tile[:],
                            rhs=pe[:, tt * H:(tt + 1) * H],
                            start=(tt == 0), stop=(tt == T - 1),
                            skip_group_check=True)
                        nc.tensor.matmul(
                            out=num[:], lhsT=s_tile[:],
                            rhs=qe[:, tt * H:(tt + 1) * H],
                            start=(tt == 0), stop=(tt == T - 1),
                            skip_group_check=True)
                    denp = drp.tile([P, H], f32, tag="denp")
                    nc.vector.tensor_scalar(
                        out=denp[:], in0=den[:], scalar1=DEN_EPS,
                        scalar2=None, op0=OP.add)
                    rden = drp.tile([P, H], f32, tag="rden")
                    nc.vector.reciprocal_approx_fast(
                        out=rden[:], in_=denp[:])
                    aggD = drp.tile([P, H], bf16, tag="aggD")
                    nc.vector.tensor_tensor(
                        out=aggD[:], in0=num[:], in1=rden[:], op=OP.mult)
                    pt = ptr.tile([P, H], bf16, space="PSUM", tag="pt")
                    for j in range(4):
                        nc.tensor.transpose(
                            out=pt[:, j * P:(j + 1) * P],
                            in_=aggD[:, j * P:(j + 1) * P],
                            identity=identb[:])
                    nc.scalar.activation(
                        out=aggT[:, :, bi * P:(bi + 1) * P],
                        in_=pt[:].rearrange("p (a b) -> p a b", a=4),
                        func=AF.Copy)
                return aggT

            def mlp1(ll, l, cchunk, aggT, w1t, xw, hpool):
                c0 = cchunk * H
                if ll == 1:
                    nc.vector.tensor_tensor(
                        out=aggT[:], in0=aggT[:],
                        in1=x1T[:, :, c0:c0 + H].rearrange("p a b -> p a b"),
                        op=OP.add)
                hs = hpool.tile([P, 8, H], bf16, tag=f"h{l}", name=f"h{l}")
                for ch in range(8):
                    hp = pall.tile([P, H], f32, space="PSUM", tag="hp")
                    if ll == 0:
                        for kt in range(2):
                            nc.tensor.matmul(
                                out=hp[:], lhsT=xw[:, kt, ch, :],
                                rhs=xt_sl[:, kt, c0:c0 + H],
                                start=(kt == 0), stop=False)
                        for kt in range(4):
                            nc.tensor.matmul(
                                out=hp[:], lhsT=w1t[:, kt, ch, :],
                                rhs=aggT[:, kt, :],
                                start=False, stop=(kt == 3))
                    else:
                        for kt in range(4):
                            nc.tensor.matmul(
                                out=hp[:], lhsT=w1t[:, kt, ch, :],
                                rhs=aggT[:, kt, :],
                                start=(kt == 0), stop=(kt == 3))
                    bidx = (ll * 2 + l) * 8 + ch
                    nc.scalar.activation(
                        out=hs[:, ch, :], in_=hp[:], func=AF.Relu,
                        bias=bias_t[:, bidx:bidx + 1])
                return hs

            def layer(ll):
                tables = sf_table if ll == 0 else [x1_table, x1_table]
                w1t = {}
                w2t = {}
                xw = {}
                for l in range(2):
                    w1t[l] = load(wpl, wd[f"w1t_{ll}_{l}"], [P, 4, 8, P],
                                  bf16, f"w1t{l}")
                    w2t[l] = load(wpl, wd[f"w2t_{ll}_{l}"], [P, 8, 4, P],
                                  bf16, f"w2t{l}")
                    if ll == 0:
                        xw[l] = load(wpl, wd[f"wft_0_{l}"], [P, 2, 8, P],
                                     bf16, f"xw{l}")
                # pass A: link 0 aggregation + MLP1 for all cchunks
                hs0 = []
                for cchunk in range(NCH):
                    aggT = agg_chunk(ll, 0, cchunk, tables[0],
                                     m_on_act=(cchunk % 2 == 0))
                    hs0.append(mlp1(ll, 0, cchunk, aggT,
                                    w1t[0], xw.get(0), h0p))
                # pass B: link 1 aggregation + MLP1 + MLP2 + outputs
                for cchunk in range(NCH):
                    c0 = cchunk * H
                    aggT = agg_chunk(ll, 1, cchunk, tables[1],
                                     m_on_act=(cchunk % 2 == 1))
                    hs1 = mlp1(ll, 1, cchunk, aggT, w1t[1], xw.get(1), h1p)
                    hs = {0: hs0[cchunk], 1: hs1}
                    if ll == 1:
                        y_sb = ysbp.tile([P, 4, H], bf16, tag="y_sb",
                                         name="y_sb")
                    for ch3 in range(4):
                        yp = pall.tile([P, H], f32, space="PSUM", tag="hp")
                        for l in range(2):
                            for kt in range(8):
                                nc.tensor.matmul(
                                    out=yp[:],
                                    lhsT=w2t[l][:, kt, ch3, :],
                                    rhs=hs[l][:, kt, :],
                                    start=(l == 0 and kt == 0),
                                    stop=(l == 1 and kt == 7))
                        if ll == 0:
                            nc.scalar.activation(
                                out=x1T[:, ch3, c0:c0 + H], in_=yp[:],
                                func=AF.Lrelu, alpha=0.01)
                        else:
                            nc.vector.tensor_copy(out=y_sb[:, ch3, :],
                                                  in_=yp[:])
                    if ll == 0:
                        for bi in range(4):
                            b = cchunk * 4 + bi
                            pt = ptr.tile([P, H], bf16, space="PSUM", tag="pt")
                            for j in range(4):
                                nc.tensor.transpose(
                                    out=pt[:, j * P:(j + 1) * P],
                                    in_=x1T[:, j,
                                            c0 + bi * P:c0 + (bi + 1) * P],
                                    identity=identb[:])
                            rows = scr.tile([P, H], bf16, tag="scr_bf")
                            nc.vector.tensor_copy(out=rows[:], in_=pt[:])
                            nc.sync.dma_start(
                                out=x1_bounce.ap()[b * P:(b + 1) * P, :],
                                in_=rows[:])
                        nc.gpsimd.collective_compute(
                            "AllGather", OP.bypass, replica_groups=RG,
                            ins=[x1_bounce.ap()[c0:c0 + H, :].opt()],
                            outs=[x1_table.ap()[cchunk * NCORES * H:
                                                (cchunk + 1) * NCORES * H,
                                                :].opt()])
                    else:
                        nc.sync.dma_start(
                            out=y_ext.ap()[cchunk, :, :],
                            in_=y_sb[:].rearrange("p a b -> p (a b)"))

            layer(0)
            layer(1)

    nc.compile()
    return nc


def _prep_weights(inputs):
    out = {}
    bias_cols = []
    for ll in range(2):
        pre = "l0" if ll == 0 else "l1"
        for l in range(2):
            w1 = np.asarray(inputs[f"{pre}_w1"][l], np.float32)   # [1024, 512]
            w2 = np.asarray(inputs[f"{pre}_w2"][l], np.float32)   # [512, 1024]
            g = np.asarray(inputs[f"{pre}_g"][l], np.float32)
            b = np.asarray(inputs[f"{pre}_b"][l], np.float32)
            m = np.asarray(inputs[f"{pre}_m"][l], np.float32)
            v = np.asarray(inputs[f"{pre}_v"][l], np.float32)
            we = np.asarray(inputs[f"{pre}_edge"][l], np.float32)[:, 0]
            s = g / np.sqrt(v + BN_EPS)
            bb = b - m * s
            w1s = s[:, None] * w1
            out[f"w1t_{ll}_{l}"] = _pack_lhst(w1s.T)
            out[f"w2t_{ll}_{l}"] = _pack_lhst(w2.T)
            out[f"we_{ll}_{l}"] = we.astype(np.float32)           # host-only
            bias_cols.append(bb.reshape(8, P).T)                  # [128, 8]
            if ll == 0:
                ws = np.asarray(inputs["l0_src"][l], np.float32)
                wdm = np.asarray(inputs["l0_dst"][l], np.float32)
                wf = s[:, None] * (w1 @ wdm)                      # [1024, 256]
                out[f"wst_{l}"] = np.ascontiguousarray(
                    ws.T.reshape(2, P, H).transpose(1, 0, 2)).astype(nbf)
                out[f"wft_0_{l}"] = _pack_lhst(wf.T)
    out["bias_all"] = np.ascontiguousarray(
        np.concatenate(bias_cols, axis=1)).astype(np.float32)     # [128, 32]
    return out


def kernel(**inputs):
    x = np.asarray(inputs["x"], np.float32)
    ei = np.asarray(inputs["ei_flat"], np.int32)
    ea = np.asarray(inputs["ea_flat"], np.float32)
    lens = (int(inputs["len0"]), int(inputs["len1"]))

    T, per_core = _host_prep(ei, ea, lens)
    wshared = _prep_weights(inputs)

    if T not in _cache:
        _cache[T] = _build(T)
    nc = _cache[T]

    in_maps = []
    for c in range(NCORES):
        core = per_core[c]
        lo = c * SHARD
        xs = x[lo:lo + SHARD]
        xs_pad = np.vstack([xs, np.zeros((1, FIN), np.float32)])
        xt2 = np.ascontiguousarray(xs_pad[core["perm"]].T)        # [256, SLOTS]
        xt_slt = np.ascontiguousarray(
            xt2.reshape(2, P, SLOTS).transpose(1, 0, 2)).astype(nbf)
        im = dict(xt_slots=xt_slt)
        for l in range(2):
            im[f"src0_{l}"] = core[f"src0_{l}"]
            im[f"src1_{l}"] = core[f"src1_{l}"]
            im[f"S_{l}"] = core[f"S_{l}"]
            # weea[b, p, tt*H+f] = ea[edge(b,tt,p)] * we[f]
            e_bpt = np.ascontiguousarray(
                core[f"eav_{l}"].transpose(0, 2, 1))      # [NBLK, P, T]
            for ll in range(2):
                we = wshared[f"we_{ll}_{l}"]
                im[f"weea_{ll}_{l}"] = np.ascontiguousarray(
                    (e_bpt[:, :, :, None] * we[None, None, None, :])
                    .reshape(NBLK, P, T * H)).astype(nbf)
        im.update({k: v for k, v in wshared.items()
                   if not k.startswith("we_")})
        in_maps.append(im)

    res = run_bass_kernel_spmd(nc, in_maps, core_ids=list(range(NCORES)))
    globals()["LAST_RESULT"] = res
    out = np.empty((N_NODES, H), np.float32)
    for c in range(NCORES):
        yc = res.results[c]["y_out"]                              # [5,128,2048]
        ysm = np.ascontiguousarray(
            yc.reshape(NCH, P, 4, H).transpose(0, 3, 2, 1)
        ).reshape(SLOTS, H)
        lo = c * SHARD
        out[lo:lo + SHARD] = ysm[per_core[c]["slot_of_node"]]
    return np.ascontiguousarray(out.astype(np.float32))


# revision 31
# speedup vs baseline: 1.0691x; 1.0691x over previous
"""GNN message-passing (GENConv-style, 2 layers x 2 link types) on 8 trn2 cores.

Sharding: partition by destination node range (2500 nodes/core). Each core owns
its nodes' incoming edges for both links/layers. All tables are kept in SLOT
space (per-core bin-packed slot order), so shard outputs are written with plain
contiguous DMAs (no indirect scatters) and AllGathered as bf16 gather tables;
gather indices are precomputed host-side in slot space. Per-block edge-tile
gathers are batched into one indirect DMA (T tiles = T*128 rows) with
compute_op=add onto a pre-filled we*ea buffer. Segment softmax num/den are
one-hot matmuls against host-built S matrices (feature-major). All matmuls run
in bf16; BN scale is folded into W1, BN bias is applied via the Relu
activation's per-partition bias operand. For layer 1 (Identity lin_dst), x1 is
added into agg before the single W1 matmul. Each layer runs as two passes
(link 0 fully, then link 1 + MLP2) so link-0 compute hides link-1's AllGather.
y is written feature-major and un-permuted on the host.
"""

import os

import numpy as np
import ml_dtypes

import concourse.bass as bass
import concourse.mybir as mybir
import concourse.tile as tile
from concourse import bacc
from concourse.bass_utils import run_bass_kernel_spmd
from concourse.masks import make_identity

N_NODES = 20000
FIN = 256
H = 512
H2 = 1024
NCORES = 8
SHARD = N_NODES // NCORES  # 2500
P = 128
NBLK = 20           # slot blocks per core (20*128 = 2560 slots >= 2500)
SLOTS = NBLK * P    # 2560
NCH = SLOTS // H    # 5 slot-chunks of 512
BN_EPS = 1e-5
DEN_EPS = 1e-20

f32 = mybir.dt.float32
bf16 = mybir.dt.bfloat16
i32 = mybir.dt.int32
i16 = mybir.dt.int16
AF = mybir.ActivationFunctionType
OP = mybir.AluOpType

nbf = np.dtype(ml_dtypes.bfloat16)

_cache = {}


def _pack_lhst(wt):
    """[K, M] -> [128, K//128, M//128, 128] so [:, kt, ch, :] is a lhsT tile."""
    K, M = wt.shape
    return np.ascontiguousarray(
        wt.reshape(K // P, P, M // P, P).transpose(1, 0, 2, 3)
    ).astype(nbf)


def _bin_pack(d0, d1):
    """Assign SHARD local nodes to NBLK blocks (<=128 nodes each), balancing
    per-link edge load. Returns list of sorted node-id arrays."""
    d_tot = d0 + d1
    order = np.argsort(-d_tot, kind="stable")
    loads = np.zeros(NBLK, dtype=np.int64)
    counts = np.zeros(NBLK, dtype=np.int64)
    blocks = [[] for _ in range(NBLK)]
    for n in order:
        cand = np.where(counts < P)[0]
        b = cand[np.argmin(loads[cand])]
        blocks[b].append(int(n))
        loads[b] += d_tot[n]
        counts[b] += 1
    return [np.array(sorted(b), dtype=np.int64) for b in blocks]


def _host_prep(ei, ea, lens):
    """Build per-core edge-structure inputs. Returns (T, per_core list)."""
    E = ei.shape[1]
    src_all = ei[0].astype(np.int64)
    dst_all = ei[1].astype(np.int64)
    link0 = np.zeros(E, dtype=bool)
    link0[: lens[0]] = True

    per_core = []
    # slot_global[n] = core(n)*SLOTS + slot_in_core(n)
    slot_global = np.full(N_NODES, -1, dtype=np.int64)
    for c in range(NCORES):
        lo, hi = c * SHARD, (c + 1) * SHARD
        core = {}
        m_core = (dst_all >= lo) & (dst_all < hi)
        dloc_all = dst_all - lo
        d0 = np.bincount(dloc_all[m_core & link0], minlength=SHARD)
        d1 = np.bincount(dloc_all[m_core & ~link0], minlength=SHARD)
        blocks = _bin_pack(d0, d1)

        slot_of_node = np.full(SHARD, -1, dtype=np.int64)
        perm = np.full(SLOTS, SHARD, dtype=np.int64)  # padding -> zero row
        for b, nodes in enumerate(blocks):
            slot_of_node[nodes] = b * P + np.arange(len(nodes))
            perm[b * P: b * P + len(nodes)] = nodes
        assert (slot_of_node >= 0).all()
        core["perm"] = perm
        core["slot_of_node"] = slot_of_node
        slot_global[lo:hi] = c * SLOTS + slot_of_node

        core["links"] = []
        for l in range(2):
            m = m_core & (link0 if l == 0 else ~link0)
            e_idx = np.nonzero(m)[0]
            e_slot = slot_of_node[dst_all[e_idx] - lo]
            e_blk = e_slot // P
            o = np.argsort(e_blk, kind="stable")
            e_idx, e_slot, e_blk = e_idx[o], e_slot[o], e_blk[o]
            cnt = np.bincount(e_blk, minlength=NBLK)
            core["links"].append((e_idx, e_slot, cnt))
        per_core.append(core)

    T = 0
    for core in per_core:
        for (_, _, cnt) in core["links"]:
            T = max(T, int(np.ceil(cnt.max() / P)))

    # x1 table row: chunk-sliced AllGather layout
    cc = slot_global % SLOTS // H            # which slot-chunk (0..4)
    corev = slot_global // SLOTS
    rr = slot_global % SLOTS % H
    x1_row = cc * (NCORES * H) + corev * H + rr

    for core in per_core:
        for l in range(2):
            e_idx, e_slot, cnt = core["links"][l]
            src0 = np.zeros((NBLK, T * P), dtype=np.int64)  # global node id
            eav = np.zeros((NBLK, T * P), dtype=np.float32)
            S = np.zeros((NBLK, T * P, P), dtype=nbf)
            off = 0
            for b in range(NBLK):
                k = int(cnt[b])
                sl = slice(off, off + k)
                src0[b, :k] = src_all[e_idx[sl]]
                eav[b, :k] = ea[e_idx[sl], 0]
                S[b, np.arange(k), e_slot[sl] - b * P] = 1.0
                off += k
            def pack16(rows):
                # rows [NBLK, T*P] in edge order -> [P, NBLK*(T*P//16)]
                # int16, idx i of block b at [i%16 (replicated x8), b*NI + i//16]
                ni = T * P // 16
                out = np.zeros((P, NBLK * ni), np.int16)
                for b in range(NBLK):
                    w = rows[b].reshape(ni, 16).T.astype(np.int16)  # [16, NI]
                    for rep in range(8):
                        out[rep * 16:(rep + 1) * 16, b * ni:(b + 1) * ni] = w
                return out

            core[f"src0_{l}"] = pack16(slot_global[src0])
            core[f"src1_{l}"] = pack16(x1_row[src0])
            core[f"ea_{l}"] = np.ascontiguousarray(
                eav.reshape(NBLK * T, P).T).astype(np.float32)
            core[f"S_{l}"] = S.reshape(NBLK * T, P, P).copy()
        del core["links"]

    return T, per_core


def _build(T):
    NT = NBLK * T
    nc = bacc.Bacc("TRN2", target_bir_lowering=False, debug=False,
                   num_devices=NCORES)

    def din(name, shape, dt):
        return nc.dram_tensor(name, shape, dt, kind="ExternalInput")

    NI16 = NBLK * (T * P // 16)
    xt_slots = din("xt_slots", [P, 2, SLOTS], bf16)
    edge_d = {}
    for l in range(2):
        edge_d[f"src0_{l}"] = din(f"src0_{l}", [P, NI16], i16)
        edge_d[f"src1_{l}"] = din(f"src1_{l}", [P, NI16], i16)
        edge_d[f"ea_{l}"] = din(f"ea_{l}", [P, NT], f32)
        edge_d[f"S_{l}"] = din(f"S_{l}", [NT, P, P], bf16)
    wst_d = [din(f"wst_{l}", [P, 2, H], bf16) for l in range(2)]
    bias_d = nc.dram_tensor("bias_all", [P, 32], f32, kind="ExternalInput")
    wd = {}
    for ll in range(2):
        for l in range(2):
            wd[f"w1t_{ll}_{l}"] = din(f"w1t_{ll}_{l}", [P, 4, 8, P], bf16)
            wd[f"w2t_{ll}_{l}"] = din(f"w2t_{ll}_{l}", [P, 8, 4, P], bf16)
            wd[f"we_{ll}_{l}"] = din(f"we_{ll}_{l}", [P, H], bf16)
    for l in range(2):
        wd[f"wft_0_{l}"] = din(f"wft_0_{l}", [P, 2, 8, P], bf16)
    y_ext = nc.dram_tensor("y_out", [NCH, P, 4 * H], bf16, kind="ExternalOutput")

    sf_bounce = [nc.dram_tensor(f"sf_bounce_{l}", [SLOTS, H], bf16)
                 for l in range(2)]
    sf_table = [nc.dram_tensor(f"sf_table_{l}", [NCORES * SLOTS, H], bf16,
                               addr_space="Shared") for l in range(2)]
    x1_bounce = nc.dram_tensor("x1_bounce", [SLOTS, H], bf16)
    x1_table = nc.dram_tensor("x1_table", [NCORES * SLOTS, H], bf16,
                              addr_space="Shared")

    RG = [list(range(NCORES))]

    with tile.TileContext(nc) as tc:
        with (
            tc.tile_pool(name="const", bufs=1) as cp,
            tc.tile_pool(name="wpool", bufs=1) as wpl,
            tc.tile_pool(name="gzp", bufs=2) as gzp,
            tc.tile_pool(name="spool", bufs=6) as sp,
            tc.tile_pool(name="edge", bufs=2) as ep,
            tc.tile_pool(name="dr", bufs=2) as drp,
            tc.tile_pool(name="agg", bufs=2) as ap_,
            tc.tile_pool(name="h0pool", bufs=NCH) as h0p,
            tc.tile_pool(name="h1pool", bufs=1) as h1p,
            tc.tile_pool(name="x1p", bufs=1) as x1p,
            tc.tile_pool(name="ysb", bufs=1) as ysbp,
            tc.tile_pool(name="scratch", bufs=3) as scr,
            tc.tile_pool(name="pseg", bufs=2, space="PSUM") as pseg,
            tc.tile_pool(name="pall", bufs=2, space="PSUM") as pall,
            tc.tile_pool(name="ptr", bufs=2, space="PSUM") as ptr,
        ):
            def load(pool, dram, shape, dt, tag):
                t = pool.tile(shape, dt, tag=tag, name=tag)
                nc.sync.dma_start(out=t[:], in_=dram.ap())
                return t

            xt_sl = load(cp, xt_slots, [P, 2, SLOTS], bf16, "xt_sl")
            src_t = {}
            ea_t = []
            for l in range(2):
                src_t[(0, l)] = load(cp, edge_d[f"src0_{l}"], [P, NI16], i16,
                                     f"src0{l}")
                src_t[(1, l)] = load(cp, edge_d[f"src1_{l}"], [P, NI16], i16,
                                     f"src1{l}")
                ea_t.append(load(cp, edge_d[f"ea_{l}"], [P, NT], f32, f"ea{l}"))
            wst_t = [load(cp, wst_d[l], [P, 2, H], bf16, f"wst{l}")
                     for l in range(2)]
            we_t = {}
            for ll in range(2):
                for l in range(2):
                    we_t[(ll, l)] = load(cp, wd[f"we_{ll}_{l}"], [P, H], bf16,
                                         f"we{ll}{l}")
            bias_t = load(cp, bias_d, [P, 32], f32, "bias_all")
            identb = cp.tile([P, P], bf16, tag="identb")
            identf = cp.tile([P, P], f32, tag="identf")
            make_identity(nc, identf[:])
            nc.vector.tensor_copy(out=identb[:], in_=identf[:])

            # ---- sf shards (slot order) + AllGather per link ----
            def sf_phase():
              for l in range(2):
                for b in range(NBLK):
                    ps = pall.tile([P, H], f32, space="PSUM", tag="hp")
                    for kt in range(2):
                        nc.tensor.matmul(
                            out=ps[:],
                            lhsT=xt_sl[:, kt, b * P:(b + 1) * P],
                            rhs=wst_t[l][:, kt, :],
                            start=(kt == 0), stop=(kt == 1))
                    sf_sb = scr.tile([P, H], bf16, tag="scr_bf")
                    nc.vector.tensor_copy(out=sf_sb[:], in_=ps[:])
                    nc.sync.dma_start(
                        out=sf_bounce[l].ap()[b * P:(b + 1) * P, :],
                        in_=sf_sb[:])
                nc.gpsimd.collective_compute(
                    "AllGather", OP.bypass, replica_groups=RG,
                    ins=[sf_bounce[l].ap().opt()],
                    outs=[sf_table[l].ap().opt()])

            sf_phase()
            x1T = x1p.tile([P, 4, SLOTS], bf16, tag="x1T")

            def agg_chunk(ll, l, cchunk, table, m_on_act):
                """Aggregate 4 blocks of cchunk for link l -> aggT tile.

                den/num are computed dst-major ([dst, feat], one stationary
                S-load + two N=512 matmuls per edge tile), then transposed
                back to feature-major via TensorE."""
                aggT = ap_.tile([P, 4, H], bf16, tag=f"aggT{l}",
                                name=f"aggT{l}")
                NI = T * P // 16
                for bi in range(4):
                    b = cchunk * 4 + bi
                    gz = gzp.tile([P, T, H], bf16, tag="gz")
                    nc.gpsimd.dma_gather(
                        gz[:], table.ap(),
                        src_t[(ll, l)][:, b * NI:(b + 1) * NI],
                        T * P, T * P, H)
                    gzf = gz[:].rearrange("p t h -> p (t h)")
                    wa = gzp.tile([P, T * H], bf16, tag="wa")
                    for tt in range(T):
                        t_idx = b * T + tt
                        nc.vector.tensor_scalar(
                            out=wa[:, tt * H:(tt + 1) * H],
                            in0=we_t[(ll, l)][:],
                            scalar1=ea_t[l][:, t_idx:t_idx + 1],
                            scalar2=None, op0=OP.mult)
                    # z = gathered + we*ea  (in place into gz)
                    nc.vector.tensor_tensor(
                        out=gzf, in0=gzf, in1=wa[:], op=OP.add)
                    m = ep.tile([P, T * H], bf16, tag="m")
                    nc.vector.tensor_scalar(
                        out=m[:], in0=gzf, scalar1=0.0,
                        scalar2=None, op0=OP.max)
                    pe = ep.tile([P, T * H], bf16, tag="pe")
                    nc.scalar.activation(out=pe[:], in_=m[:], func=AF.Exp)
                    qe = ep.tile([P, T * H], bf16, tag="qe")
                    nc.vector.tensor_tensor(
                        out=qe[:], in0=m[:], in1=pe[:], op=OP.mult)
                    den = pseg.tile([P, H], f32, space="PSUM", tag="den")
                    num = pseg.tile([P, H], f32, space="PSUM", tag="num")
                    for tt in range(T):
                        t_idx = b * T + tt
                        s_tile = sp.tile([P, P], bf16, tag="S")
                        nc.sync.dma_start(
                            out=s_tile[:],
                            in_=edge_d[f"S_{l}"][t_idx, :, :])
                        nc.tensor.matmul(
                            out=den[:], lhsT=s_tile[:],
                            rhs=pe[:, tt * H:(tt + 1) * H],
                            start=(tt == 0), stop=(tt == T - 1),
                            skip_group_check=True)
                        nc.tensor.matmul(
                            out=num[:], lhsT=s_tile[:],
                            rhs=qe[:, tt * H:(tt + 1) * H],
                            start=(tt == 0), stop=(tt == T - 1),
                            skip_group_check=True)
                    denp = drp.tile([P, H], f32, tag="denp")
                    nc.vector.tensor_scalar(
                        out=denp[:], in0=den[:], scalar1=DEN_EPS,
                        scalar2=None, op0=OP.add)
                    rden = drp.tile([P, H], f32, tag="rden")
                    nc.vector.reciprocal_approx_fast(
                        out=rden[:], in_=denp[:])
                    aggD = drp.tile([P, H], bf16, tag="aggD")
                    nc.vector.tensor_tensor(
                        out=aggD[:], in0=num[:], in1=rden[:], op=OP.mult)
                    pt = ptr.tile([P, H], bf16, space="PSUM", tag="pt")
                    for j in range(4):
                        nc.tensor.transpose(
                            out=pt[:, j * P:(j + 1) * P],
                            in_=aggD[:, j * P:(j + 1) * P],
                            identity=identb[:])
                    nc.scalar.activation(
                        out=aggT[:, :, bi * P:(bi + 1) * P],
                        in_=pt[:].rearrange("p (a b) -> p a b", a=4),
                        func=AF.Copy)
                return aggT

            def mlp1(ll, l, cchunk, aggT, w1t, xw, hpool):
                c0 = cchunk * H
                if ll == 1:
                    nc.vector.tensor_tensor(
                        out=aggT[:], in0=aggT[:],
                        in1=x1T[:, :, c0:c0 + H].rearrange("p a b -> p a b"),
                        op=OP.add)
                hs = hpool.tile([P, 8, H], bf16, tag=f"h{l}", name=f"h{l}")
                for ch in range(8):
                    hp = pall.tile([P, H], f32, space="PSUM", tag="hp")
                    if ll == 0:
                        for kt in range(2):
                            nc.tensor.matmul(
                                out=hp[:], lhsT=xw[:, kt, ch, :],
                                rhs=xt_sl[:, kt, c0:c0 + H],
                                start=(kt == 0), stop=False)
                        for kt in range(4):
                            nc.tensor.matmul(
                                out=hp[:], lhsT=w1t[:, kt, ch, :],
                                rhs=aggT[:, kt, :],
                                start=False, stop=(kt == 3))
                    else:
                        for kt in range(4):
                            nc.tensor.matmul(
                                out=hp[:], lhsT=w1t[:, kt, ch, :],
                                rhs=aggT[:, kt, :],
                                start=(kt == 0), stop=(kt == 3))
                    bidx = (ll * 2 + l) * 8 + ch
                    nc.scalar.activation(
                        out=hs[:, ch, :], in_=hp[:], func=AF.Relu,
                        bias=bias_t[:, bidx:bidx + 1])
                return hs

            def layer(ll):
                tables = sf_table if ll == 0 else [x1_table, x1_table]
                w1t = {}
                w2t = {}
                xw = {}
                for l in range(2):
                    w1t[l] = load(wpl, wd[f"w1t_{ll}_{l}"], [P, 4, 8, P],
                                  bf16, f"w1t{l}")
                    w2t[l] = load(wpl, wd[f"w2t_{ll}_{l}"], [P, 8, 4, P],
                                  bf16, f"w2t{l}")
                    if ll == 0:
                        xw[l] = load(wpl, wd[f"wft_0_{l}"], [P, 2, 8, P],
                                     bf16, f"xw{l}")
                # pass A: link 0 aggregation + MLP1 for all cchunks
                hs0 = []
                for cchunk in range(NCH):
                    aggT = agg_chunk(ll, 0, cchunk, tables[0],
                                     m_on_act=(cchunk % 2 == 0))
                    hs0.append(mlp1(ll, 0, cchunk, aggT,
                                    w1t[0], xw.get(0), h0p))
                # pass B: link 1 aggregation + MLP1 + MLP2 + outputs
                for cchunk in range(NCH):
                    c0 = cchunk * H
                    aggT = agg_chunk(ll, 1, cchunk, tables[1],
                                     m_on_act=(cchunk % 2 == 1))
                    hs1 = mlp1(ll, 1, cchunk, aggT, w1t[1], xw.get(1), h1p)
                    hs = {0: hs0[cchunk], 1: hs1}
                    if ll == 1:
                        y_sb = ysbp.tile([P, 4, H], bf16, tag="y_sb",
                                         name="y_sb")
                    for ch3 in range(4):
                        yp = pall.tile([P, H], f32, space="PSUM", tag="hp")
                        for l in range(2):
                            for kt in range(8):
                                nc.tensor.matmul(
                                    out=yp[:],
                                    lhsT=w2t[l][:, kt, ch3, :],
                                    rhs=hs[l][:, kt, :],
                                    start=(l == 0 and kt == 0),
                                    stop=(l == 1 and kt == 7))
                        if ll == 0:
                            nc.scalar.activation(
                                out=x1T[:, ch3, c0:c0 + H], in_=yp[:],
                                func=AF.Lrelu, alpha=0.01)
                        else:
                            nc.vector.tensor_copy(out=y_sb[:, ch3, :],
                                                  in_=yp[:])
                    if ll == 0:
                        for bi in range(4):
                            b = cchunk * 4 + bi
                            pt = ptr.tile([P, H], bf16, space="PSUM", tag="pt")
                            for j in range(4):
                                nc.tensor.transpose(
                                    out=pt[:, j * P:(j + 1) * P],
                                    in_=x1T[:, j,
                                            c0 + bi * P:c0 + (bi + 1) * P],
                                    identity=identb[:])
                            rows = scr.tile([P, H], bf16, tag="scr_bf")
                            nc.vector.tensor_copy(out=rows[:], in_=pt[:])
                            nc.sync.dma_start(
                                out=x1_bounce.ap()[b * P:(b + 1) * P, :],
                                in_=rows[:])
                        nc.gpsimd.collective_compute(
                            "AllGather", OP.bypass, replica_groups=RG,
                            ins=[x1_bounce.ap()[c0:c0 + H, :].opt()],
                            outs=[x1_table.ap()[cchunk * NCORES * H:
                                                (cchunk + 1) * NCORES * H,
                                                :].opt()])
                    else:
                        nc.sync.dma_start(
                            out=y_ext.ap()[cchunk, :, :],
                            in_=y_sb[:].rearrange("p a b -> p (a b)"))

            layer(0)
            layer(1)

    nc.compile()
    return nc


def _prep_weights(inputs):
    out = {}
    bias_cols = []
    for ll in range(2):
        pre = "l0" if ll == 0 else "l1"
        for l in range(2):
            w1 = np.asarray(inputs[f"{pre}_w1"][l], np.float32)   # [1024, 512]
            w2 = np.asarray(inputs[f"{pre}_w2"][l], np.float32)   # [512, 1024]
            g = np.asarray(inputs[f"{pre}_g"][l], np.float32)
            b = np.asarray(inputs[f"{pre}_b"][l], np.float32)
            m = np.asarray(inputs[f"{pre}_m"][l], np.float32)
            v = np.asarray(inputs[f"{pre}_v"][l], np.float32)
            we = np.asarray(inputs[f"{pre}_edge"][l], np.float32)[:, 0]
            s = g / np.sqrt(v + BN_EPS)
            bb = b - m * s
            w1s = s[:, None] * w1
            out[f"w1t_{ll}_{l}"] = _pack_lhst(w1s.T)
            out[f"w2t_{ll}_{l}"] = _pack_lhst(w2.T)
            out[f"we_{ll}_{l}"] = np.ascontiguousarray(
                np.broadcast_to(we.astype(nbf), (P, H)))
            bias_cols.append(bb.reshape(8, P).T)                  # [128, 8]
            if ll == 0:
                ws = np.asarray(inputs["l0_src"][l], np.float32)
                wdm = np.asarray(inputs["l0_dst"][l], np.float32)
                wf = s[:, None] * (w1 @ wdm)                      # [1024, 256]
                out[f"wst_{l}"] = np.ascontiguousarray(
                    ws.T.reshape(2, P, H).transpose(1, 0, 2)).astype(nbf)
                out[f"wft_0_{l}"] = _pack_lhst(wf.T)
    out["bias_all"] = np.ascontiguousarray(
        np.concatenate(bias_cols, axis=1)).astype(np.float32)     # [128, 32]
    return out


def kernel(**inputs):
    x = np.asarray(inputs["x"], np.float32)
    ei = np.asarray(inputs["ei_flat"], np.int32)
    ea = np.asarray(inputs["ea_flat"], np.float32)
    lens = (int(inputs["len0"]), int(inputs["len1"]))

    T, per_core = _host_prep(ei, ea, lens)
    wshared = _prep_weights(inputs)

    if T not in _cache:
        _cache[T] = _build(T)
    nc = _cache[T]

    in_maps = []
    for c in range(NCORES):
        core = per_core[c]
        lo = c * SHARD
        xs = x[lo:lo + SHARD]
        xs_pad = np.vstack([xs, np.zeros((1, FIN), np.float32)])
        xt2 = np.ascontiguousarray(xs_pad[core["perm"]].T)        # [256, SLOTS]
        xt_slt = np.ascontiguousarray(
            xt2.reshape(2, P, SLOTS).transpose(1, 0, 2)).astype(nbf)
        im = dict(xt_slots=xt_slt)
        for l in range(2):
            im[f"src0_{l}"] = core[f"src0_{l}"]
            im[f"src1_{l}"] = core[f"src1_{l}"]
            im[f"ea_{l}"] = core[f"ea_{l}"]
            im[f"S_{l}"] = core[f"S_{l}"]
        im.update(wshared)
        in_maps.append(im)

    res = run_bass_kernel_spmd(nc, in_maps, core_ids=list(range(NCORES)))
    globals()["LAST_RESULT"] = res
    out = np.empty((N_NODES, H), np.float32)
    for c in range(NCORES):
        yc = res.results[c]["y_out"]                              # [5,128,2048]
        ysm = np.ascontiguousarray(
            yc.reshape(NCH, P, 4, H).transpose(0, 3, 2, 1)
        ).reshape(SLOTS, H)
        lo = c * SHARD
        out[lo:lo + SHARD] = ysm[per_core[c]["slot_of_node"]]
    return np.ascontiguousarray(out.astype(np.float32))
